# revision 28
# baseline (speedup 1.0000x reference)
"""Trainium2 Bass kernel for nn_MultiHeadAttention_27711128994021.

Reference math (faithful to the oracle, including its independent-sum einsum):
  q = x@Wq.T+bq ; k = x@Wk.T+bk ; v = x@Wv.T+bv       (B,S,H,D)
  rq, rk = rope(pos, q, k)
  phi_q = elu(rq)+1 ; phi_k = (elu(rk)+1) * notpad
  attn[b,s,h,v] = z[b,h,s] * (sum_q phi_q[b,s,h,q]) * (sum_k kv[b,h,v,k])
    with kv = einsum("bshv,bshk->bhvk", v, phi_k), z = 1/clip(phi_q . k_sum)
  out = attn @ Wo.T + bo

Attention is rank-1 per (b,h) (q and k independently summed), so the V
projection collapses to kvsum = Wv @ (psk.T @ x).T + bv*psktot and the out
projection to rank-17: y = [zq|1] @ [Wo2; bo].

Sharding: 8 cores = (batch b, seq half). Cross-core data: all-reduce (pairs)
of xk=psk.T@x [16,1024], psktot [16], ksum [1024] (~70KB).

v2 layout strategy (vs the v1 DMA-heavy setup):
  - ALL large operands arrive from the host pre-transposed / pre-permuted /
    pre-cast to bf16 (xT, x natural, Wq/Wk row-permuted + transposed, Wv/Wo
    transposed, cos/sin/mask broadcast tiles) — zero on-device transposes
    or dtype casts in the critical path; compute starts ~5us in.
  - K path is j-outer with xk partial matmuls per j-chunk so the collective
    fires immediately at K end.
  - Q path phi chunks are kept in SBUF; everything that depends on the
    collective (den/qsum reductions, kvsum, Wo2, y) runs in a tail emitted
    after all Q projections, so the ~45us collective latency hides under
    the Q-path compute instead of stalling the statically-ordered tensor
    queue.
  - Collective results are re-read from DRAM with reshaped access patterns
    (xkT gather, psktot row) instead of on-chip transposes.
  - y is written bf16 and widened to f32 on the host.
"""

import functools

import numpy as np
import ml_dtypes

import concourse.bass as bass
import concourse.mybir as mybir
import concourse.tile as tile
from concourse import bacc
from concourse.bass_utils import run_bass_kernel_spmd

F32 = mybir.dt.float32
BF16 = mybir.dt.bfloat16
AF = mybir.ActivationFunctionType
ALU = mybir.AluOpType

P = 128
B, S, H, D = 4, 4096, 16, 64
DM = H * D            # 1024
SC = 2048             # seq rows per core
KT = DM // P          # 8 contraction tiles
FT = DM // P          # 8 feature tiles (2 heads each)
NJ = SC // 512        # 4 s-chunks of 512
NST = SC // P         # 16 seq tiles of 128
EPS = 1e-6
N_CORES = 8
CC_XK, CC_PT, CC_KS = H * DM, H, P * FT
CC_LEN = CC_XK + CC_PT + CC_KS

bf = ml_dtypes.bfloat16


def build_program(collective=True):
    nc = bacc.Bacc(
        "TRN2", target_bir_lowering=False, debug=False, num_devices=N_CORES
    )

    # ---- I/O (all heavy tensors host-prepared: bf16, transposed, permuted) ----
    xT_in = nc.dram_tensor("xT", [DM, SC], BF16, kind="ExternalInput").ap()
    xn_in = nc.dram_tensor("xn", [SC, DM], BF16, kind="ExternalInput").ap()
    wqT_in = nc.dram_tensor("wqT", [DM, DM], BF16, kind="ExternalInput").ap()
    wkT_in = nc.dram_tensor("wkT", [DM, DM], BF16, kind="ExternalInput").ap()
    wvT_in = nc.dram_tensor("wvT", [DM, DM], BF16, kind="ExternalInput").ap()
    woT_in = nc.dram_tensor("woT", [DM, DM], BF16, kind="ExternalInput").ap()
    cosb_in = nc.dram_tensor("cosb", [P, SC], BF16, kind="ExternalInput").ap()
    sinb_in = nc.dram_tensor("sinb", [P, SC], BF16, kind="ExternalInput").ap()
    cosbm_in = nc.dram_tensor("cosbm", [P, SC], BF16, kind="ExternalInput").ap()
    sinbm_in = nc.dram_tensor("sinbm", [P, SC], BF16, kind="ExternalInput").ap()
    mb_in = nc.dram_tensor("mb", [P, SC], BF16, kind="ExternalInput").ap()
    bqT_in = nc.dram_tensor("bqT", [P, FT], F32, kind="ExternalInput").ap()
    bkT_in = nc.dram_tensor("bkT", [P, FT], F32, kind="ExternalInput").ap()
    bvb_in = nc.dram_tensor("bvb", [1, DM], BF16, kind="ExternalInput").ap()
    bob_in = nc.dram_tensor("bob", [1, DM], BF16, kind="ExternalInput").ap()
    ident_in = nc.dram_tensor("ident", [P, P], BF16, kind="ExternalInput").ap()
    psign_in = nc.dram_tensor("psign", [P, P], BF16, kind="ExternalInput").ap()
    selk_in = nc.dram_tensor("selk", [P, 2], BF16, kind="ExternalInput").ap()
    selq0_in = nc.dram_tensor("selq0", [P, FT * 4], BF16, kind="ExternalInput").ap()
    ones_in = nc.dram_tensor("onescol", [P, 1], BF16, kind="ExternalInput").ap()
    npad_in = nc.dram_tensor("npadc", [P, 1], F32, kind="ExternalInput").ap()
    y_out = nc.dram_tensor("y", [SC, DM], BF16, kind="ExternalOutput").ap()

    with tile.TileContext(nc) as tc:
        with (
            tc.tile_pool(name="const", bufs=1) as cp,
            tc.tile_pool(name="work", bufs=3) as wp,
            tc.tile_pool(name="phip", bufs=32) as php,
            tc.tile_pool(name="xnp", bufs=6) as xp,
            tc.tile_pool(name="pA", bufs=2, space="PSUM") as pA,
            tc.tile_pool(name="pB", bufs=2, space="PSUM") as pB,
            tc.tile_pool(name="pC", bufs=2, space="PSUM") as pC,
            tc.tile_pool(name="pD", bufs=2, space="PSUM") as pD,
            tc.tile_pool(name="dram", bufs=1, space="DRAM") as dp,
        ):
            cc_i = dp.tile([CC_LEN], F32, tag="cc_i")
            cc_o = dp.tile([CC_LEN], F32, tag="cc_o")

            # ---------------- input loads ----------------
            ident = cp.tile([P, P], BF16, tag="ident")
            psign = cp.tile([P, P], BF16, tag="psign")
            selk = cp.tile([P, 2], BF16, tag="selk")
            selq = cp.tile([P, FT, 4], BF16, tag="selq")
            onescol = cp.tile([P, 1], BF16, tag="onescol")
            npadc = cp.tile([P, 1], F32, tag="npadc")
            bqT = cp.tile([P, FT], F32, tag="bqT")
            bkT = cp.tile([P, FT], F32, tag="bkT")
            bvb = cp.tile([1, DM], BF16, tag="bvb")
            wo2ext = cp.tile([H + 1, DM], BF16, tag="wo2ext")
            nc.sync.dma_start(ident[:], ident_in)
            nc.sync.dma_start(psign[:], psign_in)
            nc.sync.dma_start(selk[:], selk_in)
            nc.sync.dma_start(
                selq[:].rearrange("p t m -> p (t m)"), selq0_in
            )
            nc.sync.dma_start(onescol[:], ones_in)
            nc.sync.dma_start(npadc[:], npad_in)
            nc.sync.dma_start(bqT[:], bqT_in)
            nc.sync.dma_start(bkT[:], bkT_in)
            nc.sync.dma_start(bvb[:], bvb_in)
            nc.sync.dma_start(wo2ext[H:H + 1, :], bob_in)

            cosbm = cp.tile([P, SC], BF16, tag="cosbm")
            sinbm = cp.tile([P, SC], BF16, tag="sinbm")
            m_b = cp.tile([P, SC], BF16, tag="m_b")
            cosb = cp.tile([P, SC], BF16, tag="cosb")
            sinb = cp.tile([P, SC], BF16, tag="sinb")
            nc.gpsimd.dma_start(cosbm[:], cosbm_in)
            nc.gpsimd.dma_start(sinbm[:], sinbm_in)
            nc.gpsimd.dma_start(m_b[:], mb_in)

            # Startup-critical set first (wk + xT chunk 0 + masked cos/sin),
            # spread across queues so descriptor-gen isn't the serializer;
            # everything else streams behind at lower priority.
            wkTs = cp.tile([P, KT, DM], BF16, tag="wkTs")
            wqTs = cp.tile([P, KT, DM], BF16, tag="wqTs")
            wvTs = cp.tile([P, KT, DM], BF16, tag="wvTs")
            woTs = cp.tile([P, KT, DM], BF16, tag="woTs")
            xTs = cp.tile([P, KT, SC], BF16, tag="xTs")

            def big_w(queue, dst, src):
                queue.dma_start(
                    dst[:], src.rearrange("(kt p) d -> p kt d", p=P)
                )

            # wk arrives in per-t column slices so the first K chunk can
            # start after ~1.3MB instead of the whole working set.
            for t in range(FT):
                tsl = slice(t * P, (t + 1) * P)
                nc.sync.dma_start(
                    wkTs[:, :, tsl],
                    wkT_in[:, tsl].rearrange("(kt p) c -> p kt c", p=P),
                )
            for kt in range(KT):
                nc.sync.dma_start(
                    xTs[:, kt, 0:512], xT_in[kt * P:(kt + 1) * P, 0:512]
                )
            for kt in range(KT):
                nc.sync.dma_start(
                    xTs[:, kt, 512:SC], xT_in[kt * P:(kt + 1) * P, 512:SC]
                )
            # Same queue => FIFO at the HW DMA ring: these heavy loads only
            # transfer after the startup-critical set above has landed.
            big_w(nc.sync, wqTs, wqT_in)
            nc.sync.dma_start(cosb[:], cosb_in)
            nc.sync.dma_start(sinb[:], sinb_in)
            big_w(nc.sync, wvTs, wvT_in)
            big_w(nc.sync, woTs, woT_in)

            # accumulators
            psk_nat = cp.tile([P, NST, H], BF16, tag="psk_nat")
            ksum_parts = cp.tile([P, FT, NJ, 2], F32, tag="ksum_parts")
            qd_nat = cp.tile([P, FT, NST * 4], F32, tag="qd_nat")
            xk_acc = cp.tile([H, DM], F32, tag="xk_acc")
            pt_acc = cp.tile([H, 1], F32, tag="pt_acc")
            zqext = cp.tile([H + 1, SC], BF16, tag="zqext")
            nc.vector.memset(zqext[:], 1.0)
            kvsel = cp.tile([P, KT, H], BF16, tag="kvsel")
            nc.vector.memset(kvsel[:], 0.0)

            # ---------------- K path (j-outer) ----------------
            for j in range(NJ):
                jsl = slice(j * 512, (j + 1) * 512)
                xn_tiles = []
                for sub in range(4):
                    st = 4 * j + sub
                    xnt = xp.tile([P, DM], BF16, tag="xn")
                    nc.gpsimd.dma_start(
                        xnt[:], xn_in[st * P:(st + 1) * P, :]
                    )
                    xn_tiles.append(xnt)
                for t in range(FT):
                    tsl = slice(t * P, (t + 1) * P)
                    projP = pA.tile([P, 512], F32, tag="projP")
                    for kt in range(KT):
                        nc.tensor.matmul(
                            projP[:], wkTs[:, kt, tsl], xTs[:, kt, jsl],
                            start=(kt == 0), stop=(kt == KT - 1),
                        )
                    s1 = wp.tile([P, 512], BF16, tag="s1")
                    s2 = wp.tile([P, 512], BF16, tag="s2")
                    nc.vector.scalar_tensor_tensor(
                        s1[:], projP[:], bkT[:, t:t + 1], cosbm[:, jsl],
                        ALU.add, ALU.mult,
                    )
                    nc.vector.scalar_tensor_tensor(
                        s2[:], projP[:], bkT[:, t:t + 1], sinbm[:, jsl],
                        ALU.add, ALU.mult,
                    )
                    ropeP = pB.tile([P, 512], F32, tag="ropeP")
                    nc.tensor.matmul(ropeP[:], psign[:], s2[:])
                    rope = wp.tile([P, 512], BF16, tag="rope")
                    nc.vector.tensor_tensor(rope[:], ropeP[:], s1[:], ALU.add)
                    # min(exp(x),1) == exp(-relu(-x)): clamp via two ACTs, so
                    # the pool-side assembly stays add/mult-only. ksum rides
                    # the ACT accum_outs: sum(e1*m_b) == sum(e1) - npad since
                    # e1 == 1 exactly on pad columns; npad corrects at the end.
                    rneg = wp.tile([P, 512], BF16, tag="e")
                    nc.scalar.activation(rneg[:], rope[:], AF.Relu, scale=-1.0)
                    e1 = wp.tile([P, 512], BF16, tag="e")
                    nc.scalar.activation(
                        e1[:], rneg[:], AF.Exp, scale=-1.0,
                        accum_out=ksum_parts[:, t, j, 0:1],
                    )
                    r = wp.tile([P, 512], BF16, tag="s2")
                    nc.scalar.activation(
                        r[:], rope[:], AF.Relu,
                        accum_out=ksum_parts[:, t, j, 1:2],
                    )
                    em1 = wp.tile([P, 512], BF16, tag="s1")
                    nc.gpsimd.tensor_tensor(em1[:], e1[:], m_b[:, jsl], ALU.mult)
                    phik = wp.tile([P, 512], BF16, tag="phik")
                    nc.gpsimd.tensor_tensor(phik[:], em1[:], r[:], ALU.add)
                    pskP = pC.tile([P, 8], F32, tag="small")
                    for sub in range(4):
                        nc.tensor.matmul(
                            pskP[:, 2 * sub:2 * sub + 2],
                            phik[:, sub * P:(sub + 1) * P],
                            selk[:],
                        )
                    nc.scalar.copy(
                        psk_nat[:, 4 * j:4 * j + 4, 2 * t:2 * t + 2],
                        pskP.rearrange("p (sub hh) -> p sub hh", hh=2),
                    )

                # xk / psktot partial matmuls for this j-chunk
                xkP1 = pD.tile([H, 512], F32, tag="xkP")
                xkP2 = pD.tile([H, 512], F32, tag="xkP")
                ptP = pC.tile([H, 1], F32, tag="small")
                for sub in range(4):
                    st = 4 * j + sub
                    fl = (sub == 0)
                    ll = (sub == 3)
                    nc.tensor.matmul(
                        xkP1[:], psk_nat[:, st, :], xn_tiles[sub][:, 0:512],
                        start=fl, stop=ll,
                    )
                    nc.tensor.matmul(
                        xkP2[:], psk_nat[:, st, :], xn_tiles[sub][:, 512:DM],
                        start=fl, stop=ll,
                    )
                    nc.tensor.matmul(
                        ptP[:], psk_nat[:, st, :], onescol[:],
                        start=fl, stop=ll,
                    )
                if j == 0:
                    nc.scalar.copy(xk_acc[:, 0:512], xkP1[:])
                    nc.scalar.copy(xk_acc[:, 512:DM], xkP2[:])
                    nc.scalar.copy(pt_acc[:], ptP[:])
                else:
                    nc.vector.tensor_tensor(
                        xk_acc[:, 0:512], xk_acc[:, 0:512], xkP1[:], ALU.add
                    )
                    nc.vector.tensor_tensor(
                        xk_acc[:, 512:DM], xk_acc[:, 512:DM], xkP2[:], ALU.add
                    )
                    nc.vector.tensor_tensor(
                        pt_acc[:], pt_acc[:], ptP[:], ALU.add
                    )

            # ---------------- collective ----------------
            kst1 = cp.tile([P, FT, 2], F32, tag="kst1")
            kst2 = cp.tile([P, FT, 2], F32, tag="kst2")
            kst3 = cp.tile([P, FT], F32, tag="kst3")
            ksum_flat = cp.tile([P, FT], F32, tag="ksum_flat")
            nc.vector.tensor_tensor(
                kst1[:], ksum_parts[:, :, 0, :], ksum_parts[:, :, 1, :], ALU.add
            )
            nc.vector.tensor_tensor(
                kst2[:], ksum_parts[:, :, 2, :], ksum_parts[:, :, 3, :], ALU.add
            )
            nc.vector.tensor_tensor(kst1[:], kst1[:], kst2[:], ALU.add)
            nc.vector.tensor_tensor(
                kst3[:], kst1[:, :, 0], kst1[:, :, 1], ALU.add
            )
            # subtract the pad-column count (e1 contributes exactly 1 there)
            nc.vector.tensor_scalar(
                ksum_flat[:], kst3[:], npadc[:, 0:1], None, ALU.subtract
            )
            with nc.allow_non_contiguous_dma(reason="70KB collective bundle"):
                nc.sync.dma_start(
                    cc_i[0:CC_XK].rearrange("(a b) -> a b", a=H), xk_acc[:]
                )
                nc.sync.dma_start(
                    cc_i[CC_XK:CC_XK + CC_PT].rearrange("(a b) -> a b", a=H),
                    pt_acc[:],
                )
                nc.sync.dma_start(
                    cc_i[CC_XK + CC_PT:CC_LEN].rearrange("(a b) -> a b", a=P),
                    ksum_flat[:],
                )
            if collective:
                nc.gpsimd.collective_compute(
                    "AllReduce",
                    ALU.add,
                    replica_groups=[[0, 1], [2, 3], [4, 5], [6, 7]],
                    ins=[cc_i.opt()],
                    outs=[cc_o.opt()],
                )
            else:  # timing-model variant: TimelineSim can't model collectives
                nc.sync.dma_start(cc_o[:], cc_i[:])

            # unpack DMAs (fast, contiguous); their consumers are all emitted
            # in the tail so they never block the Q-path engine queues.
            ksum_r = cp.tile([P, FT], F32, tag="ksum_r")
            xk_rf = cp.tile([H, DM], F32, tag="xk_rf")
            ptrow_f = cp.tile([1, H], F32, tag="ptrow_f")
            with nc.allow_non_contiguous_dma(reason="70KB collective bundle"):
                nc.sync.dma_start(
                    ksum_r[:],
                    cc_o[CC_XK + CC_PT:CC_LEN].rearrange("(a b) -> a b", a=P),
                )
                nc.sync.dma_start(
                    xk_rf[:], cc_o[0:CC_XK].rearrange("(a b) -> a b", a=H)
                )
                nc.sync.dma_start(
                    ptrow_f[:],
                    cc_o[CC_XK:CC_XK + CC_PT].rearrange("(a b) -> a b", a=1),
                )

            # ---------------- Q path (phi chunks stored for the tail) -----
            phiq_tiles = []
            for j in range(NJ):
                jsl = slice(j * 512, (j + 1) * 512)
                for t in range(FT):
                    tsl = slice(t * P, (t + 1) * P)
                    projP = pA.tile([P, 512], F32, tag="projP")
                    for kt in range(KT):
                        nc.tensor.matmul(
                            projP[:], wqTs[:, kt, tsl], xTs[:, kt, jsl],
                            start=(kt == 0), stop=(kt == KT - 1),
                        )
                    s1 = wp.tile([P, 512], BF16, tag="s1")
                    s2 = wp.tile([P, 512], BF16, tag="s2")
                    nc.vector.scalar_tensor_tensor(
                        s1[:], projP[:], bqT[:, t:t + 1], cosb[:, jsl],
                        ALU.add, ALU.mult,
                    )
                    nc.vector.scalar_tensor_tensor(
                        s2[:], projP[:], bqT[:, t:t + 1], sinb[:, jsl],
                        ALU.add, ALU.mult,
                    )
                    ropeP = pB.tile([P, 512], F32, tag="ropeP")
                    nc.tensor.matmul(ropeP[:], psign[:], s2[:])
                    rope = wp.tile([P, 512], BF16, tag="rope")
                    nc.vector.tensor_tensor(rope[:], ropeP[:], s1[:], ALU.add)
                    rneg = wp.tile([P, 512], BF16, tag="e")
                    nc.scalar.activation(rneg[:], rope[:], AF.Relu, scale=-1.0)
                    e1 = wp.tile([P, 512], BF16, tag="e")
                    nc.scalar.activation(e1[:], rneg[:], AF.Exp, scale=-1.0)
                    r = wp.tile([P, 512], BF16, tag="s2")
                    nc.scalar.activation(r[:], rope[:], AF.Relu)
                    phiq = php.tile([P, 512], BF16, tag="phiq")
                    nc.gpsimd.tensor_tensor(phiq[:], e1[:], r[:], ALU.add)
                    phiq_tiles.append(phiq)

            # ---------------- tail: kvsum / Wo2, qd, z, y ----------------
            # collective unpack consumers (vector/scalar/tensor) live here
            ptrow = cp.tile([1, H], BF16, tag="ptrow")
            nc.vector.tensor_copy(ptrow[:], ptrow_f[:])
            for t in range(FT):
                nc.vector.tensor_copy(
                    selq[0:64, t, 2:3], ksum_r[0:64, t:t + 1]
                )
                nc.vector.tensor_copy(
                    selq[64:P, t, 3:4], ksum_r[64:P, t:t + 1]
                )
            xk_rb = cp.tile([H, DM], BF16, tag="xk_rb")
            nc.vector.tensor_copy(xk_rb[:], xk_rf[:])
            xkT = cp.tile([P, KT, H], BF16, tag="xkT")
            for kt in range(KT):
                xkTP = pC.tile([P, H], BF16, tag="small")
                nc.tensor.transpose(
                    xkTP[:], xk_rb[:, kt * P:(kt + 1) * P], ident[0:H, 0:H]
                )
                nc.scalar.copy(xkT[:, kt, :], xkTP[:])
            # kvsum in [h, v] orientation; bias rides the accumulation.
            kvsb = cp.tile([H, DM], BF16, tag="kvsb")
            for half in range(2):
                hsl = slice(half * 512, (half + 1) * 512)
                kvP = pD.tile([H, 512], F32, tag="xkP")
                for kt in range(KT):
                    nc.tensor.matmul(
                        kvP[:], xkT[:, kt, :], wvTs[:, kt, hsl],
                        start=(kt == 0), stop=False,
                    )
                nc.tensor.matmul(
                    kvP[:], ptrow[:], bvb[:, hsl], start=False, stop=True
                )
                nc.scalar.copy(kvsb[:, hsl], kvP[:])
            # kvsel: per v-tile, keep only the owning head's column
            for kt in range(KT):
                kvT = pC.tile([P, H], BF16, tag="small")
                nc.tensor.transpose(
                    kvT[:], kvsb[:, kt * P:(kt + 1) * P], ident[0:H, 0:H]
                )
                nc.scalar.copy(
                    kvsel[0:64, kt, 2 * kt:2 * kt + 1],
                    kvT[0:64, 2 * kt:2 * kt + 1],
                )
                nc.scalar.copy(
                    kvsel[64:P, kt, 2 * kt + 1:2 * kt + 2],
                    kvT[64:P, 2 * kt + 1:2 * kt + 2],
                )
            for half in range(2):
                hsl = slice(half * 512, (half + 1) * 512)
                w2P = pD.tile([H, 512], F32, tag="xkP")
                for kt in range(KT):
                    nc.tensor.matmul(
                        w2P[:], kvsel[:, kt, :], woTs[:, kt, hsl],
                        start=(kt == 0), stop=(kt == KT - 1),
                    )
                nc.scalar.copy(wo2ext[0:H, hsl], w2P[:])

            # qd reductions + z + y, per j-chunk
            qdv = qd_nat.rearrange("p t (st m) -> p st t m", m=4)
            den_c = cp.tile([P, 256], F32, tag="den_c")
            dcv = den_c.rearrange("p (st t hh) -> p st t hh", st=NST, t=FT)
            den_cl = cp.tile([P, 256], F32, tag="den_cl")
            zr = cp.tile([P, 256], F32, tag="zr")
            zq_c = cp.tile([P, 256], BF16, tag="zq_c")
            zqv = zq_c.rearrange("p (st t hh) -> p st t hh", st=NST, t=FT)
            zrv = zr.rearrange("p (st t hh) -> p st t hh", st=NST, t=FT)
            for j in range(NJ):
                for t in range(FT):
                    phiq = phiq_tiles[j * FT + t]
                    qdP = pC.tile([P, 16], F32, tag="small")
                    for sub in range(4):
                        nc.tensor.matmul(
                            qdP[:, 4 * sub:4 * sub + 4],
                            phiq[:, sub * P:(sub + 1) * P],
                            selq[:, t, :],
                        )
                    nc.scalar.copy(qd_nat[:, t, 16 * j:16 * (j + 1)], qdP[:])
                zsl = slice(64 * j, 64 * (j + 1))
                sts = slice(4 * j, 4 * (j + 1))
                nc.vector.tensor_copy(dcv[:, sts], qdv[:, sts, :, 2:4])
                nc.vector.tensor_scalar_max(den_cl[:, zsl], den_c[:, zsl], EPS)
                nc.vector.reciprocal(zr[:, zsl], den_cl[:, zsl])
                nc.vector.tensor_tensor(
                    zqv[:, sts], zrv[:, sts], qdv[:, sts, :, 0:2], ALU.mult
                )
                for sub in range(4):
                    st = 4 * j + sub
                    ssl = slice(st * P, (st + 1) * P)
                    zP = pC.tile([H, P], BF16, tag="small")
                    nc.tensor.transpose(
                        zP[:], zq_c[:, st * H:(st + 1) * H], ident[:]
                    )
                    nc.scalar.copy(zqext[0:H, ssl], zP[:])
                    for half in range(2):
                        hsl = slice(half * 512, (half + 1) * 512)
                        yP = pB.tile([P, 512], F32, tag="ropeP")
                        nc.tensor.matmul(yP[:], zqext[:, ssl], wo2ext[:, hsl])
                        ysb = wp.tile([P, 512], BF16, tag="ysb")
                        if half == 0:
                            nc.vector.tensor_copy(ysb[:], yP[:])
                        else:
                            nc.scalar.copy(ysb[:], yP[:])
                        nc.gpsimd.dma_start(y_out[ssl, hsl], ysb[:])

    nc.finalize()
    return nc


def _consts():
    ident = np.eye(P, dtype=bf)
    psign = np.zeros((P, P), np.float32)
    for h in range(2):
        for i in range(32):
            psign[h * 64 + 32 + i, h * 64 + i] = -1.0   # even' = .. - s*odd
            psign[h * 64 + i, h * 64 + 32 + i] = 1.0    # odd'  = .. + s*even
    selk = np.zeros((P, 2), np.float32)
    selk[0:64, 0] = 1.0
    selk[64:P, 1] = 1.0
    selq0 = np.zeros((P, FT, 4), np.float32)
    selq0[0:64, :, 0] = 1.0
    selq0[64:P, :, 1] = 1.0
    onescol = np.ones((P, 1), np.float32)
    return {
        "ident": ident,
        "psign": psign.astype(bf),
        "selk": selk.astype(bf),
        "selq0": np.ascontiguousarray(selq0.reshape(P, FT * 4).astype(bf)),
        "onescol": onescol.astype(bf),
    }


# permutation: new feature row h*64 + pr*32 + i  <-  old row h*64 + 2*i + pr
def _perm_idx():
    idx = np.zeros(DM, np.int64)
    for h in range(H):
        for pr in range(2):
            for i in range(32):
                idx[h * 64 + pr * 32 + i] = h * 64 + 2 * i + pr
    return idx


@functools.lru_cache(maxsize=1)
def _program():
    return build_program()


def make_in_maps(inputs):
    consts = _consts()
    perm = _perm_idx()
    Wq = np.asarray(inputs["Wq"], np.float32)
    Wk = np.asarray(inputs["Wk"], np.float32)
    Wv = np.asarray(inputs["Wv"], np.float32)
    Wo = np.asarray(inputs["Wo"], np.float32)
    shared = {
        "wqT": np.ascontiguousarray(Wq[perm].T.astype(bf)),
        "wkT": np.ascontiguousarray(Wk[perm].T.astype(bf)),
        "wvT": np.ascontiguousarray(Wv.T.astype(bf)),
        "woT": np.ascontiguousarray(Wo.T.astype(bf)),
        "bqT": np.ascontiguousarray(
            np.asarray(inputs["bq"], np.float32)[perm].reshape(FT, P).T
        ),
        "bkT": np.ascontiguousarray(
            np.asarray(inputs["bk"], np.float32)[perm].reshape(FT, P).T
        ),
        "bvb": np.asarray(inputs["bv"], np.float32).reshape(1, DM).astype(bf),
        "bob": np.asarray(inputs["bo"], np.float32).reshape(1, DM).astype(bf),
        **consts,
    }
    x = np.asarray(inputs["x"], np.float32)
    pos = np.asarray(inputs["rotary_pos_enc"], np.float32)   # (S, 1, D)
    mask = np.asarray(inputs["padding_mask"], np.int32)
    rowsel = np.arange(P) % 32
    in_maps = []
    for c in range(N_CORES):
        b, hf = c // 2, c % 2
        sl = slice(hf * SC, (hf + 1) * SC)
        xc = x[b, sl].astype(bf)
        posc = pos[sl, 0, :]                                  # (SC, 64)
        sinr = np.ascontiguousarray(posc[:, 0:32].T)          # (32, SC)
        cosr = np.ascontiguousarray(posc[:, 32:64].T)
        cosb = cosr[rowsel]                                   # (P, SC)
        sinb = sinr[rowsel]
        notpad = (mask[b, sl] == 0).astype(np.float32)        # (SC,)
        in_maps.append(
            {
                "xT": np.ascontiguousarray(xc.T),
                "xn": np.ascontiguousarray(xc),
                "cosb": cosb.astype(bf),
                "sinb": sinb.astype(bf),
                "cosbm": (cosb * notpad).astype(bf),
                "sinbm": (sinb * notpad).astype(bf),
                "mb": np.ascontiguousarray(
                    np.broadcast_to(notpad, (P, SC))
                ).astype(bf),
                "npadc": np.full((P, 1), SC - notpad.sum(), np.float32),
                **shared,
            }
        )
    return in_maps


def run(inputs, **kwargs):
    nc = _program()
    in_maps = make_in_maps(inputs)
    res = run_bass_kernel_spmd(
        nc, in_maps, core_ids=list(range(N_CORES)), **kwargs
    )
    out = np.zeros((B, S, DM), np.float32)
    for c in range(N_CORES):
        b, hf = c // 2, c % 2
        out[b, hf * SC:(hf + 1) * SC, :] = res.results[c]["y"].astype(
            np.float32
        )
    return out, res


def kernel(**inputs) -> np.ndarray:
    out, _ = run(inputs)
    return out


# revision 31
# speedup vs baseline: 1.6705x; 1.6705x over previous
"""Trainium2 Bass kernel for nn_MultiHeadAttention_27711128994021.

Reference math (faithful to the oracle, including its independent-sum einsum):
  q = x@Wq.T+bq ; k = x@Wk.T+bk ; v = x@Wv.T+bv       (B,S,H,D)
  rq, rk = rope(pos, q, k)
  phi_q = elu(rq)+1 ; phi_k = (elu(rk)+1) * notpad
  attn[b,s,h,v] = z[b,h,s] * (sum_q phi_q[b,s,h,q]) * (sum_k kv[b,h,v,k])
    with kv = einsum("bshv,bshk->bhvk", v, phi_k), z = 1/clip(phi_q . k_sum)
  out = attn @ Wo.T + bo

Attention is rank-1 per (b,h) (q and k independently summed), so the V
projection collapses to kvsum = Wv @ (psk.T @ x).T + bv*psktot and the out
projection to rank-17: y = [zq|1] @ [Wo2; bo].

Sharding: 8 cores = (batch b, seq half). Cross-core data: all-reduce (pairs)
of xk=psk.T@x [16,1024], psktot [16], ksum [1024] (~70KB).

v2 layout strategy (vs the v1 DMA-heavy setup):
  - ALL large operands arrive from the host pre-transposed / pre-permuted /
    pre-cast to bf16 (xT, x natural, Wq/Wk row-permuted + transposed, Wv/Wo
    transposed, cos/sin/mask broadcast tiles) — zero on-device transposes
    or dtype casts in the critical path; compute starts ~5us in.
  - K path is j-outer with xk partial matmuls per j-chunk so the collective
    fires immediately at K end.
  - Q path phi chunks are kept in SBUF; everything that depends on the
    collective (den/qsum reductions, kvsum, Wo2, y) runs in a tail emitted
    after all Q projections, so the ~45us collective latency hides under
    the Q-path compute instead of stalling the statically-ordered tensor
    queue.
  - Collective results are re-read from DRAM with reshaped access patterns
    (xkT gather, psktot row) instead of on-chip transposes.
  - y is written bf16 and widened to f32 on the host.
"""

import functools

import numpy as np
import ml_dtypes

import concourse.bass as bass
import concourse.mybir as mybir
import concourse.tile as tile
from concourse import bacc
from concourse.bass_utils import run_bass_kernel_spmd

F32 = mybir.dt.float32
BF16 = mybir.dt.bfloat16
AF = mybir.ActivationFunctionType
ALU = mybir.AluOpType

P = 128
B, S, H, D = 4, 4096, 16, 64
DM = H * D            # 1024
SC = 2048             # seq rows per core
KT = DM // P          # 8 contraction tiles
FT = DM // P          # 8 feature tiles (2 heads each)
NJ = SC // 512        # 4 s-chunks of 512
NST = SC // P         # 16 seq tiles of 128
EPS = 1e-6
N_CORES = 8
CC_XK, CC_PT, CC_KS = H * DM, H, P * FT
CC_LEN = CC_XK + CC_PT + CC_KS

bf = ml_dtypes.bfloat16


def build_program(collective=True):
    nc = bacc.Bacc(
        "TRN2", target_bir_lowering=False, debug=False, num_devices=N_CORES
    )

    # ---- I/O (all heavy tensors host-prepared: bf16, transposed, permuted) ----
    xT_in = nc.dram_tensor("xT", [DM, SC], BF16, kind="ExternalInput").ap()
    xn_in = nc.dram_tensor("xn", [SC, DM], BF16, kind="ExternalInput").ap()
    wqT_in = nc.dram_tensor("wqT", [DM, DM], BF16, kind="ExternalInput").ap()
    wkT_in = nc.dram_tensor("wkT", [DM, DM], BF16, kind="ExternalInput").ap()
    wvT_in = nc.dram_tensor("wvT", [DM, DM], BF16, kind="ExternalInput").ap()
    woT_in = nc.dram_tensor("woT", [DM, DM], BF16, kind="ExternalInput").ap()
    cosb_in = nc.dram_tensor("cosb", [P, SC], BF16, kind="ExternalInput").ap()
    sinb_in = nc.dram_tensor("sinb", [P, SC], BF16, kind="ExternalInput").ap()
    cosbm_in = nc.dram_tensor("cosbm", [P, SC], BF16, kind="ExternalInput").ap()
    sinbm_in = nc.dram_tensor("sinbm", [P, SC], BF16, kind="ExternalInput").ap()
    mb_in = nc.dram_tensor("mb", [P, SC], BF16, kind="ExternalInput").ap()
    bqT_in = nc.dram_tensor("bqT", [P, FT], F32, kind="ExternalInput").ap()
    bkT_in = nc.dram_tensor("bkT", [P, FT], F32, kind="ExternalInput").ap()
    bvb_in = nc.dram_tensor("bvb", [1, DM], BF16, kind="ExternalInput").ap()
    bob_in = nc.dram_tensor("bob", [1, DM], BF16, kind="ExternalInput").ap()
    ident_in = nc.dram_tensor("ident", [P, P], BF16, kind="ExternalInput").ap()
    psign_in = nc.dram_tensor("psign", [P, P], BF16, kind="ExternalInput").ap()
    selk_in = nc.dram_tensor("selk", [P, 2], BF16, kind="ExternalInput").ap()
    selq0_in = nc.dram_tensor("selq0", [P, FT * 4], BF16, kind="ExternalInput").ap()
    ones_in = nc.dram_tensor("onescol", [P, 1], BF16, kind="ExternalInput").ap()
    y_out = nc.dram_tensor("y", [SC, DM], BF16, kind="ExternalOutput").ap()

    with tile.TileContext(nc) as tc:
        with (
            tc.tile_pool(name="const", bufs=1) as cp,
            tc.tile_pool(name="work", bufs=3) as wp,
            tc.tile_pool(name="phip", bufs=32) as php,
            tc.tile_pool(name="xnp", bufs=6) as xp,
            tc.tile_pool(name="pA", bufs=2, space="PSUM") as pA,
            tc.tile_pool(name="pB", bufs=2, space="PSUM") as pB,
            tc.tile_pool(name="pC", bufs=2, space="PSUM") as pC,
            tc.tile_pool(name="pD", bufs=2, space="PSUM") as pD,
            tc.tile_pool(name="dram", bufs=1, space="DRAM") as dp,
        ):
            cc_i = dp.tile([CC_LEN], F32, tag="cc_i")
            cc_o = dp.tile([CC_LEN], F32, tag="cc_o")

            # ---------------- input loads ----------------
            ident = cp.tile([P, P], BF16, tag="ident")
            psign = cp.tile([P, P], BF16, tag="psign")
            selk = cp.tile([P, 2], BF16, tag="selk")
            selq = cp.tile([P, FT, 4], BF16, tag="selq")
            onescol = cp.tile([P, 1], BF16, tag="onescol")
            bqT = cp.tile([P, FT], F32, tag="bqT")
            bkT = cp.tile([P, FT], F32, tag="bkT")
            bvb = cp.tile([1, DM], BF16, tag="bvb")
            wo2ext = cp.tile([H + 1, DM], BF16, tag="wo2ext")
            nc.sync.dma_start(ident[:], ident_in)
            nc.sync.dma_start(psign[:], psign_in)
            nc.sync.dma_start(selk[:], selk_in)
            nc.sync.dma_start(
                selq[:].rearrange("p t m -> p (t m)"), selq0_in
            )
            nc.sync.dma_start(onescol[:], ones_in)
            nc.sync.dma_start(bqT[:], bqT_in)
            nc.sync.dma_start(bkT[:], bkT_in)
            nc.sync.dma_start(bvb[:], bvb_in)
            nc.sync.dma_start(wo2ext[H:H + 1, :], bob_in)

            cosbm = cp.tile([P, SC], BF16, tag="cosbm")
            sinbm = cp.tile([P, SC], BF16, tag="sinbm")
            m_b = cp.tile([P, SC], BF16, tag="m_b")
            cosb = cp.tile([P, SC], BF16, tag="cosb")
            sinb = cp.tile([P, SC], BF16, tag="sinb")
            nc.gpsimd.dma_start(cosbm[:], cosbm_in)
            nc.gpsimd.dma_start(sinbm[:], sinbm_in)
            nc.gpsimd.dma_start(m_b[:], mb_in)

            # Startup-critical set first (wk + xT chunk 0 + masked cos/sin),
            # spread across queues so descriptor-gen isn't the serializer;
            # everything else streams behind at lower priority.
            wkTs = cp.tile([P, KT, DM], BF16, tag="wkTs")
            wqTs = cp.tile([P, KT, DM], BF16, tag="wqTs")
            wvTs = cp.tile([P, KT, DM], BF16, tag="wvTs")
            woTs = cp.tile([P, KT, DM], BF16, tag="woTs")
            xTs = cp.tile([P, KT, SC], BF16, tag="xTs")

            def big_w(queue, dst, src):
                queue.dma_start(
                    dst[:], src.rearrange("(kt p) d -> p kt d", p=P)
                )

            # wk arrives in per-t column slices so the first K chunk can
            # start after ~1.3MB instead of the whole working set.
            for t in range(FT):
                tsl = slice(t * P, (t + 1) * P)
                nc.sync.dma_start(
                    wkTs[:, :, tsl],
                    wkT_in[:, tsl].rearrange("(kt p) c -> p kt c", p=P),
                )
            for kt in range(KT):
                nc.sync.dma_start(
                    xTs[:, kt, 0:512], xT_in[kt * P:(kt + 1) * P, 0:512]
                )
            for kt in range(KT):
                nc.sync.dma_start(
                    xTs[:, kt, 512:SC], xT_in[kt * P:(kt + 1) * P, 512:SC]
                )
            # Same queue => FIFO at the HW DMA ring: these heavy loads only
            # transfer after the startup-critical set above has landed.
            big_w(nc.sync, wqTs, wqT_in)
            nc.sync.dma_start(cosb[:], cosb_in)
            nc.sync.dma_start(sinb[:], sinb_in)
            big_w(nc.sync, wvTs, wvT_in)
            big_w(nc.sync, woTs, woT_in)

            # accumulators
            psk_nat = cp.tile([P, NST, H], BF16, tag="psk_nat")
            ksum_parts = cp.tile([P, FT, NJ], F32, tag="ksum_parts")
            qd_nat = cp.tile([P, FT, NST * 4], F32, tag="qd_nat")
            xk_acc = cp.tile([H, DM], F32, tag="xk_acc")
            pt_acc = cp.tile([H, 1], F32, tag="pt_acc")
            zqext = cp.tile([H + 1, SC], BF16, tag="zqext")
            nc.vector.memset(zqext[:], 1.0)
            kvsel = cp.tile([P, KT, H], BF16, tag="kvsel")
            nc.vector.memset(kvsel[:], 0.0)

            # ---------------- K path (j-outer) ----------------
            for j in range(NJ):
                jsl = slice(j * 512, (j + 1) * 512)
                xn_tiles = []
                for sub in range(4):
                    st = 4 * j + sub
                    xnt = xp.tile([P, DM], BF16, tag="xn")
                    nc.sync.dma_start(
                        xnt[:], xn_in[st * P:(st + 1) * P, :]
                    )
                    xn_tiles.append(xnt)
                for t in range(FT):
                    tsl = slice(t * P, (t + 1) * P)
                    projP = pA.tile([P, 512], F32, tag="projP")
                    for kt in range(KT):
                        nc.tensor.matmul(
                            projP[:], wkTs[:, kt, tsl], xTs[:, kt, jsl],
                            start=(kt == 0), stop=(kt == KT - 1),
                        )
                    ck = wp.tile([P, 512], BF16, tag="ck")
                    nc.scalar.activation(
                        ck[:], projP[:], AF.Identity, bias=bkT[:, t:t + 1]
                    )
                    s1 = wp.tile([P, 512], BF16, tag="s1")
                    s2 = wp.tile([P, 512], BF16, tag="s2")
                    nc.vector.tensor_tensor(s1[:], ck[:], cosbm[:, jsl], ALU.mult)
                    nc.vector.tensor_tensor(s2[:], ck[:], sinbm[:, jsl], ALU.mult)
                    ropeP = pB.tile([P, 512], F32, tag="ropeP")
                    nc.tensor.matmul(ropeP[:], ident[:], s1[:], start=True, stop=False)
                    nc.tensor.matmul(ropeP[:], psign[:], s2[:], start=False, stop=True)
                    e = wp.tile([P, 512], BF16, tag="e")
                    nc.scalar.activation(e[:], ropeP[:], AF.Exp)
                    r = wp.tile([P, 512], BF16, tag="s2")
                    nc.scalar.activation(r[:], ropeP[:], AF.Relu)
                    e2 = wp.tile([P, 512], BF16, tag="s1")
                    nc.vector.tensor_tensor(e2[:], e[:], m_b[:, jsl], ALU.min)
                    phik = wp.tile([P, 512], BF16, tag="phik")
                    nc.vector.scalar_tensor_tensor(
                        phik[:], e2[:], 0.0, r[:], ALU.add, ALU.add,
                        accum_out=ksum_parts[:, t, j:j + 1],
                    )
                    pskP = pC.tile([P, 8], F32, tag="small")
                    for sub in range(4):
                        nc.tensor.matmul(
                            pskP[:, 2 * sub:2 * sub + 2],
                            phik[:, sub * P:(sub + 1) * P],
                            selk[:],
                        )
                    nc.scalar.copy(
                        psk_nat[:, 4 * j:4 * j + 4, 2 * t:2 * t + 2],
                        pskP.rearrange("p (sub hh) -> p sub hh", hh=2),
                    )

                # xk / psktot partial matmuls for this j-chunk
                xkP1 = pD.tile([H, 512], F32, tag="xkP")
                xkP2 = pD.tile([H, 512], F32, tag="xkP")
                ptP = pC.tile([H, 1], F32, tag="small")
                for sub in range(4):
                    st = 4 * j + sub
                    fl = (sub == 0)
                    ll = (sub == 3)
                    nc.tensor.matmul(
                        xkP1[:], psk_nat[:, st, :], xn_tiles[sub][:, 0:512],
                        start=fl, stop=ll,
                    )
                    nc.tensor.matmul(
                        xkP2[:], psk_nat[:, st, :], xn_tiles[sub][:, 512:DM],
                        start=fl, stop=ll,
                    )
                    nc.tensor.matmul(
                        ptP[:], psk_nat[:, st, :], onescol[:],
                        start=fl, stop=ll,
                    )
                if j == 0:
                    nc.scalar.copy(xk_acc[:, 0:512], xkP1[:])
                    nc.scalar.copy(xk_acc[:, 512:DM], xkP2[:])
                    nc.scalar.copy(pt_acc[:], ptP[:])
                else:
                    nc.vector.tensor_tensor(
                        xk_acc[:, 0:512], xk_acc[:, 0:512], xkP1[:], ALU.add
                    )
                    nc.vector.tensor_tensor(
                        xk_acc[:, 512:DM], xk_acc[:, 512:DM], xkP2[:], ALU.add
                    )
                    nc.vector.tensor_tensor(
                        pt_acc[:], pt_acc[:], ptP[:], ALU.add
                    )

            # ---------------- collective ----------------
            kst1 = cp.tile([P, FT], F32, tag="kst1")
            kst2 = cp.tile([P, FT], F32, tag="kst2")
            ksum_flat = cp.tile([P, FT], F32, tag="ksum_flat")
            nc.vector.tensor_tensor(
                kst1[:], ksum_parts[:, :, 0], ksum_parts[:, :, 1], ALU.add
            )
            nc.vector.tensor_tensor(
                kst2[:], ksum_parts[:, :, 2], ksum_parts[:, :, 3], ALU.add
            )
            nc.vector.tensor_tensor(ksum_flat[:], kst1[:], kst2[:], ALU.add)
            with nc.allow_non_contiguous_dma(reason="70KB collective bundle"):
                nc.sync.dma_start(
                    cc_i[0:CC_XK].rearrange("(a b) -> a b", a=H), xk_acc[:]
                )
                nc.sync.dma_start(
                    cc_i[CC_XK:CC_XK + CC_PT].rearrange("(a b) -> a b", a=H),
                    pt_acc[:],
                )
                nc.sync.dma_start(
                    cc_i[CC_XK + CC_PT:CC_LEN].rearrange("(a b) -> a b", a=P),
                    ksum_flat[:],
                )
            if collective:
                nc.gpsimd.collective_compute(
                    "AllReduce",
                    ALU.add,
                    replica_groups=[[0, 1], [2, 3], [4, 5], [6, 7]],
                    ins=[cc_i.opt()],
                    outs=[cc_o.opt()],
                )
            else:  # timing-model variant: TimelineSim can't model collectives
                nc.sync.dma_start(cc_o[:], cc_i[:])

            # unpack DMAs (fast, contiguous); their consumers are all emitted
            # in the tail so they never block the Q-path engine queues.
            ksum_r = cp.tile([P, FT], F32, tag="ksum_r")
            xk_rf = cp.tile([H, DM], F32, tag="xk_rf")
            ptrow_f = cp.tile([1, H], F32, tag="ptrow_f")
            with nc.allow_non_contiguous_dma(reason="70KB collective bundle"):
                nc.sync.dma_start(
                    ksum_r[:],
                    cc_o[CC_XK + CC_PT:CC_LEN].rearrange("(a b) -> a b", a=P),
                )
                nc.sync.dma_start(
                    xk_rf[:], cc_o[0:CC_XK].rearrange("(a b) -> a b", a=H)
                )
                nc.sync.dma_start(
                    ptrow_f[:],
                    cc_o[CC_XK:CC_XK + CC_PT].rearrange("(a b) -> a b", a=1),
                )

            # ---------------- Q path (phi chunks stored for the tail) -----
            phiq_tiles = []
            for j in range(NJ):
                jsl = slice(j * 512, (j + 1) * 512)
                for t in range(FT):
                    tsl = slice(t * P, (t + 1) * P)
                    projP = pA.tile([P, 512], F32, tag="projP")
                    for kt in range(KT):
                        nc.tensor.matmul(
                            projP[:], wqTs[:, kt, tsl], xTs[:, kt, jsl],
                            start=(kt == 0), stop=(kt == KT - 1),
                        )
                    ck = wp.tile([P, 512], BF16, tag="ck")
                    nc.scalar.activation(
                        ck[:], projP[:], AF.Identity, bias=bqT[:, t:t + 1]
                    )
                    s1 = wp.tile([P, 512], BF16, tag="s1")
                    s2 = wp.tile([P, 512], BF16, tag="s2")
                    nc.vector.tensor_tensor(s1[:], ck[:], cosb[:, jsl], ALU.mult)
                    nc.vector.tensor_tensor(s2[:], ck[:], sinb[:, jsl], ALU.mult)
                    ropeP = pB.tile([P, 512], F32, tag="ropeP")
                    nc.tensor.matmul(ropeP[:], ident[:], s1[:], start=True, stop=False)
                    nc.tensor.matmul(ropeP[:], psign[:], s2[:], start=False, stop=True)
                    e = wp.tile([P, 512], BF16, tag="e")
                    nc.scalar.activation(e[:], ropeP[:], AF.Exp)
                    r = wp.tile([P, 512], BF16, tag="s2")
                    nc.vector.tensor_scalar_max(r[:], ropeP[:], 0.0)
                    phiq = php.tile([P, 512], BF16, tag="phiq")
                    nc.vector.scalar_tensor_tensor(
                        phiq[:], e[:], 1.0, r[:], ALU.min, ALU.add
                    )
                    phiq_tiles.append(phiq)

            # ---------------- tail: kvsum / Wo2, qd, z, y ----------------
            # collective unpack consumers (vector/scalar/tensor) live here
            ptrow = cp.tile([1, H], BF16, tag="ptrow")
            nc.vector.tensor_copy(ptrow[:], ptrow_f[:])
            for t in range(FT):
                nc.vector.tensor_copy(
                    selq[0:64, t, 2:3], ksum_r[0:64, t:t + 1]
                )
                nc.vector.tensor_copy(
                    selq[64:P, t, 3:4], ksum_r[64:P, t:t + 1]
                )
            xk_rb = cp.tile([H, DM], BF16, tag="xk_rb")
            nc.vector.tensor_copy(xk_rb[:], xk_rf[:])
            xkT = cp.tile([P, KT, H], BF16, tag="xkT")
            for kt in range(KT):
                xkTP = pC.tile([P, H], BF16, tag="small")
                nc.tensor.transpose(
                    xkTP[:], xk_rb[:, kt * P:(kt + 1) * P], ident[0:H, 0:H]
                )
                nc.scalar.copy(xkT[:, kt, :], xkTP[:])
            # kvsum in [h, v] orientation; bias rides the accumulation.
            kvsb = cp.tile([H, DM], BF16, tag="kvsb")
            for half in range(2):
                hsl = slice(half * 512, (half + 1) * 512)
                kvP = pD.tile([H, 512], F32, tag="xkP")
                for kt in range(KT):
                    nc.tensor.matmul(
                        kvP[:], xkT[:, kt, :], wvTs[:, kt, hsl],
                        start=(kt == 0), stop=False,
                    )
                nc.tensor.matmul(
                    kvP[:], ptrow[:], bvb[:, hsl], start=False, stop=True
                )
                nc.scalar.copy(kvsb[:, hsl], kvP[:])
            # kvsel: per v-tile, keep only the owning head's column
            for kt in range(KT):
                kvT = pC.tile([P, H], BF16, tag="small")
                nc.tensor.transpose(
                    kvT[:], kvsb[:, kt * P:(kt + 1) * P], ident[0:H, 0:H]
                )
                nc.scalar.copy(
                    kvsel[0:64, kt, 2 * kt:2 * kt + 1],
                    kvT[0:64, 2 * kt:2 * kt + 1],
                )
                nc.scalar.copy(
                    kvsel[64:P, kt, 2 * kt + 1:2 * kt + 2],
                    kvT[64:P, 2 * kt + 1:2 * kt + 2],
                )
            for half in range(2):
                hsl = slice(half * 512, (half + 1) * 512)
                w2P = pD.tile([H, 512], F32, tag="xkP")
                for kt in range(KT):
                    nc.tensor.matmul(
                        w2P[:], kvsel[:, kt, :], woTs[:, kt, hsl],
                        start=(kt == 0), stop=(kt == KT - 1),
                    )
                nc.scalar.copy(wo2ext[0:H, hsl], w2P[:])

            # qd reductions + z + y, per j-chunk
            qdv = qd_nat.rearrange("p t (st m) -> p st t m", m=4)
            den_c = cp.tile([P, 256], F32, tag="den_c")
            dcv = den_c.rearrange("p (st t hh) -> p st t hh", st=NST, t=FT)
            den_cl = cp.tile([P, 256], F32, tag="den_cl")
            zr = cp.tile([P, 256], F32, tag="zr")
            zq_c = cp.tile([P, 256], BF16, tag="zq_c")
            zqv = zq_c.rearrange("p (st t hh) -> p st t hh", st=NST, t=FT)
            zrv = zr.rearrange("p (st t hh) -> p st t hh", st=NST, t=FT)
            for j in range(NJ):
                for t in range(FT):
                    phiq = phiq_tiles[j * FT + t]
                    qdP = pC.tile([P, 16], F32, tag="small")
                    for sub in range(4):
                        nc.tensor.matmul(
                            qdP[:, 4 * sub:4 * sub + 4],
                            phiq[:, sub * P:(sub + 1) * P],
                            selq[:, t, :],
                        )
                    nc.scalar.copy(qd_nat[:, t, 16 * j:16 * (j + 1)], qdP[:])
                zsl = slice(64 * j, 64 * (j + 1))
                sts = slice(4 * j, 4 * (j + 1))
                nc.vector.tensor_copy(dcv[:, sts], qdv[:, sts, :, 2:4])
                nc.vector.tensor_scalar_max(den_cl[:, zsl], den_c[:, zsl], EPS)
                nc.vector.reciprocal(zr[:, zsl], den_cl[:, zsl])
                nc.vector.tensor_tensor(
                    zqv[:, sts], zrv[:, sts], qdv[:, sts, :, 0:2], ALU.mult
                )
                for sub in range(4):
                    st = 4 * j + sub
                    ssl = slice(st * P, (st + 1) * P)
                    zP = pC.tile([H, P], BF16, tag="small")
                    nc.tensor.transpose(
                        zP[:], zq_c[:, st * H:(st + 1) * H], ident[:]
                    )
                    nc.scalar.copy(zqext[0:H, ssl], zP[:])
                    for half in range(2):
                        hsl = slice(half * 512, (half + 1) * 512)
                        yP = pB.tile([P, 512], F32, tag="ropeP")
                        nc.tensor.matmul(yP[:], zqext[:, ssl], wo2ext[:, hsl])
                        ysb = wp.tile([P, 512], BF16, tag="ysb")
                        if half == 0:
                            nc.vector.tensor_copy(ysb[:], yP[:])
                        else:
                            nc.scalar.copy(ysb[:], yP[:])
                        nc.gpsimd.dma_start(y_out[ssl, hsl], ysb[:])

    nc.finalize()
    return nc


def _consts():
    ident = np.eye(P, dtype=bf)
    psign = np.zeros((P, P), np.float32)
    for h in range(2):
        for i in range(32):
            psign[h * 64 + 32 + i, h * 64 + i] = -1.0   # even' = .. - s*odd
            psign[h * 64 + i, h * 64 + 32 + i] = 1.0    # odd'  = .. + s*even
    selk = np.zeros((P, 2), np.float32)
    selk[0:64, 0] = 1.0
    selk[64:P, 1] = 1.0
    selq0 = np.zeros((P, FT, 4), np.float32)
    selq0[0:64, :, 0] = 1.0
    selq0[64:P, :, 1] = 1.0
    onescol = np.ones((P, 1), np.float32)
    return {
        "ident": ident,
        "psign": psign.astype(bf),
        "selk": selk.astype(bf),
        "selq0": np.ascontiguousarray(selq0.reshape(P, FT * 4).astype(bf)),
        "onescol": onescol.astype(bf),
    }


# permutation: new feature row h*64 + pr*32 + i  <-  old row h*64 + 2*i + pr
def _perm_idx():
    idx = np.zeros(DM, np.int64)
    for h in range(H):
        for pr in range(2):
            for i in range(32):
                idx[h * 64 + pr * 32 + i] = h * 64 + 2 * i + pr
    return idx


@functools.lru_cache(maxsize=1)
def _program():
    return build_program()


def make_in_maps(inputs):
    consts = _consts()
    perm = _perm_idx()
    Wq = np.asarray(inputs["Wq"], np.float32)
    Wk = np.asarray(inputs["Wk"], np.float32)
    Wv = np.asarray(inputs["Wv"], np.float32)
    Wo = np.asarray(inputs["Wo"], np.float32)
    shared = {
        "wqT": np.ascontiguousarray(Wq[perm].T.astype(bf)),
        "wkT": np.ascontiguousarray(Wk[perm].T.astype(bf)),
        "wvT": np.ascontiguousarray(Wv.T.astype(bf)),
        "woT": np.ascontiguousarray(Wo.T.astype(bf)),
        "bqT": np.ascontiguousarray(
            np.asarray(inputs["bq"], np.float32)[perm].reshape(FT, P).T
        ),
        "bkT": np.ascontiguousarray(
            np.asarray(inputs["bk"], np.float32)[perm].reshape(FT, P).T
        ),
        "bvb": np.asarray(inputs["bv"], np.float32).reshape(1, DM).astype(bf),
        "bob": np.asarray(inputs["bo"], np.float32).reshape(1, DM).astype(bf),
        **consts,
    }
    x = np.asarray(inputs["x"], np.float32)
    pos = np.asarray(inputs["rotary_pos_enc"], np.float32)   # (S, 1, D)
    mask = np.asarray(inputs["padding_mask"], np.int32)
    rowsel = np.arange(P) % 32
    in_maps = []
    for c in range(N_CORES):
        b, hf = c // 2, c % 2
        sl = slice(hf * SC, (hf + 1) * SC)
        xc = x[b, sl].astype(bf)
        posc = pos[sl, 0, :]                                  # (SC, 64)
        sinr = np.ascontiguousarray(posc[:, 0:32].T)          # (32, SC)
        cosr = np.ascontiguousarray(posc[:, 32:64].T)
        cosb = cosr[rowsel]                                   # (P, SC)
        sinb = sinr[rowsel]
        notpad = (mask[b, sl] == 0).astype(np.float32)        # (SC,)
        in_maps.append(
            {
                "xT": np.ascontiguousarray(xc.T),
                "xn": np.ascontiguousarray(xc),
                "cosb": cosb.astype(bf),
                "sinb": sinb.astype(bf),
                "cosbm": (cosb * notpad).astype(bf),
                "sinbm": (sinb * notpad).astype(bf),
                "mb": np.ascontiguousarray(
                    np.broadcast_to(notpad, (P, SC))
                ).astype(bf),
                **shared,
            }
        )
    return in_maps


def run(inputs, **kwargs):
    nc = _program()
    in_maps = make_in_maps(inputs)
    res = run_bass_kernel_spmd(
        nc, in_maps, core_ids=list(range(N_CORES)), **kwargs
    )
    out = np.zeros((B, S, DM), np.float32)
    for c in range(N_CORES):
        b, hf = c // 2, c % 2
        out[b, hf * SC:(hf + 1) * SC, :] = res.results[c]["y"].astype(
            np.float32
        )
    return out, res


def kernel(**inputs) -> np.ndarray:
    out, _ = run(inputs)
    return out
